# revision 47
# baseline (speedup 1.0000x reference)
"""Alias-free activation (StyleGAN3-style) Trainium2 Bass kernel, v2.

Pipeline per image: bias-add -> 2x zero-insert upsample + 12x12 FIR (pad 10,
gain 4) -> leaky-relu(0.2)*sqrt(2) [clamp +-256 provably inactive: |Y|<=5.1]
-> 12x12 FIR + 2x downsample.

Strategy (pure data parallel, 8 images -> 8 cores):
- UP conv on TensorE as banded-Toeplitz matmuls over x rows, bf16, with TWO
  (tail: THREE) column taps packed per stream via host-prebuilt col-shifted
  replica tensors stacked along the contraction partitions. Bias and the
  4*sqrt(2)*up^2 gain are folded host-side (into x replicas / up weights).
- Leaky-relu is algebraically split: P = Relu(16*yhat) on ScalarE and
  N = Relu(-16*yhat) ((x*-16) max 0) on VectorE, both cast to fp8e4m3 and
  written interleaved as the two DoubleRow slots of the down conv.
- DOWN conv on TensorE in fp8 DoubleRow (0.5 cyc/row): slot0 multiplies P by
  1024*fd, slot1 multiplies N by -0.2*1024*fd; all three row-tiles place
  their bands at stationary column offset p0 and accumulate into ONE
  128-partition PSUM per 16-col q-chunk, so each block needs a single
  128-partition store (q-split for overlap). The down tail packs col-tap
  pairs (dd, dd+3) via a v-shifted SBUF replica of the chunk-3 P/N tile.
  ACT evacuates PSUM->SBUF with a 2^-14 output scale (GPSIMD cannot touch
  PSUM on hardware). Per block, chunk 3 is computed first so its replica
  pieces overlap the rest of the up phase. Measured end-to-end rel err
  ~9.5e-3 on hardware (gate 2e-2).
"""
import numpy as np
import ml_dtypes

import concourse.bass as bass
import concourse.bacc as bacc
import concourse.tile as tile
from concourse import mybir
from concourse.bass_utils import run_bass_kernel_spmd

N_CORES = 8
C, H, W = 128, 128, 128
QO = 127            # output spatial size
CB = 32             # channels per block
NBLK = C // CB
CG = 3              # channels per up matmul group (3*132 = 396 <= 512 psum)
A_GAIN = 4.0 * float(np.sqrt(2.0))   # up^2 * leaky sqrt(2) gain
SW = 1024.0          # down fp8 weight scale (power of 2)
OUT_SCALE = 1.0 / (16.0 * SW)
XW = 138            # padded x width (5 each side)

# (k0, rows, m0, h, pack): up chunk of interleaved Y rows [k0, k0+rows),
# contracting x rows [m0, m0+h), `pack` col-taps stacked on partitions.
UP_CHUNKS = [
    (0, 116, 0, 59, 2),
    (106, 116, 48, 64, 2),
    (210, 54, 100, 28, 3),
]
# (ci, K, p0, M, pk): down tile reading pn chunk ci rows [0, K), writing Z
# rows [p0, p0+M).  t0 = UP_CHUNKS[ci].k0.  pk=2: col-taps (dd, dd+3) are
# partition-packed via the v-shifted replica in partitions [K/2, K).
DN_TILES = [
    (0, 116, 0, 53, 1),
    (1, 114, 53, 52, 1),
    (2, 108, 105, 22, 2),
]
F32 = mybir.dt.float32
F32R = mybir.dt.float32r
BF16 = mybir.dt.bfloat16
FP8 = mybir.dt.float8e4
NP_BF16 = ml_dtypes.bfloat16
NP_E4 = ml_dtypes.float8_e4m3


def _up_streams(b, pack):
    base = -5 if b == 0 else -4
    return list(range(base, base + 6, pack))


def _batches():
    """Channel-group batches within a block: [[(c0, ncg), (c0, ncg)], ...]."""
    groups = []
    c0 = 0
    while c0 < CB:
        groups.append((c0, min(CG, CB - c0)))
        c0 += CG
    return [groups[i:i + 2] for i in range(0, len(groups), 2)]


def _build_up_weights(fu):
    """Per-chunk stationary banks [pack*h, 2, ns, rows] (bf16, gain folded)."""
    fua = fu.astype(np.float64) * A_GAIN
    out = []
    for (k0, rows, m0, h, pack) in UP_CHUNKS:
        ns = len(_up_streams(0, pack))
        w = np.zeros((pack * h, 2, ns, rows), np.float64)
        kl = np.arange(rows)[None, :]
        rr = np.arange(h)[:, None]
        frow = 2 * (m0 + rr) - (k0 + kl) + 10      # [h, rows]
        fvalid = (frow >= 0) & (frow <= 11)
        frc = np.clip(frow, 0, 11)
        for b in (0, 1):
            for si, dv0 in enumerate(_up_streams(b, pack)):
                for cp in range(pack):
                    fcol = 2 * (dv0 + cp) - b + 10
                    assert 0 <= fcol <= 11
                    w[cp * h:(cp + 1) * h, b, si, :] = np.where(
                        fvalid, fua[frc, fcol], 0.0)
        out.append(w.astype(NP_BF16))
    return out


def _build_dn_weights(fd):
    """Per-tile DoubleRow banks [K, 12, 2, 128] (fp8): slot0 = SW*fd,
    slot1 = -0.2*SW*fd for the N = relu(-yhat) branch.  The band sits at
    stationary columns [p0, p0+M) so all three tiles accumulate into one
    128-partition PSUM (Z row p = partition p)."""
    fdq = fd.astype(np.float64)
    out = []
    for (ci, K, p0, M, pk) in DN_TILES:
        t0 = UP_CHUNKS[ci][0]
        kh = 54 if pk == 2 else K
        cp_base = [0, 54]
        nst = 6 // pk
        w = np.zeros((K, 2 * nst, 2, 128), np.float64)
        r = np.arange(kh)[:, None]
        pg = np.arange(p0, p0 + M)[None, :]
        dr = (t0 + r) - 2 * pg                     # [kh, M]
        valid = (dr >= 0) & (dr <= 11)
        drc = np.clip(dr, 0, 11)
        for bp in (0, 1):
            for dd in range(nst):
                idx = bp * nst + dd
                for cp in range(pk):
                    b0_ = cp_base[cp] if pk == 2 else 0
                    band = np.where(valid, fdq[drc, 2 * (dd + 3 * cp) + bp],
                                    0.0)
                    w[b0_:b0_ + kh, idx, 0, p0:p0 + M] = SW * band
                    w[b0_:b0_ + kh, idx, 1, p0:p0 + M] = -0.2 * SW * band
        out.append(w.astype(NP_E4))
    return out


def _build_x_replicas(xb):
    """xb: [C, H, W] fp32 bias-folded image -> per-chunk replica tensors
    [pack*h, C, XW] bf16; copy cp holds x rows shifted by cp columns."""
    out = []
    for (k0, rows, m0, h, pack) in UP_CHUNKS:
        xr = np.zeros((pack * h, C, XW), np.float32)
        src = np.transpose(xb[:, m0:m0 + h, :], (1, 0, 2))  # [h, C, W]
        for cp in range(pack):
            xr[cp * h:(cp + 1) * h, :, 5 - cp:133 - cp] = src
        out.append(xr.astype(NP_BF16))
    return out


def _ap(base, dims):
    """Manual AP with base's tensor/offset and explicit [stride, count] dims."""
    return bass.AP(tensor=base.tensor, offset=base.offset, ap=dims)


def _build_program() -> bacc.Bacc:
    nc = bacc.Bacc("TRN2", target_bir_lowering=False, debug=False,
                   num_devices=N_CORES)
    xr_d = [nc.dram_tensor(f"xr{i}", [p * h, C, XW], BF16,
                           kind="ExternalInput")
            for i, (k0, rows, m0, h, p) in enumerate(UP_CHUNKS)]
    ns_l = [len(_up_streams(0, p)) for (_, _, _, _, p) in UP_CHUNKS]
    m_l = [rows for (k0, rows, m0, h, p) in UP_CHUNKS]
    wu_d = [nc.dram_tensor(f"wu{i}", [p * h, 2, ns_l[i], m_l[i]], BF16,
                           kind="ExternalInput")
            for i, (k0, rows, m0, h, p) in enumerate(UP_CHUNKS)]
    wd_d = [nc.dram_tensor(f"wd{i}", [K, 2 * (6 // pk), 2, 128], FP8,
                           kind="ExternalInput")
            for i, (ci, K, p0, M, pk) in enumerate(DN_TILES)]
    out_d = nc.dram_tensor("out", [C, QO, QO], F32, kind="ExternalOutput")

    with tile.TileContext(nc) as tc:
        with (
            tc.tile_pool(name="consts", bufs=1) as consts,
            tc.tile_pool(name="xin", bufs=3) as xin,
            tc.tile_pool(name="pn", bufs=1) as pnp,
            tc.tile_pool(name="stg", bufs=2) as stgp,
            tc.tile_pool(name="pup", bufs=3, space="PSUM") as pup,
            tc.tile_pool(name="pdn", bufs=2, space="PSUM") as pdn,
        ):
            # first chunk's input + weights first so PE starts ASAP; then the
            # rest. All loads upfront: the SP queue then only ever waits on
            # xin buffer recycling (prefetch), never behind the out-store
            # DMAs' stg waits.
            wu_sb = [None] * 3
            wd_sb = [None] * 3
            xr_blk = [[None] * 3 for _ in range(NBLK)]

            def load_wu(i):
                (k0, rows, m0, h, p) = UP_CHUNKS[i]
                t = consts.tile([128, 2, ns_l[i], m_l[i]], BF16,
                                name=f"wu{i}", tag=f"wu{i}")
                nc.sync.dma_start(out=t[:p * h], in_=wu_d[i][:, :, :, :])
                wu_sb[i] = t

            def load_xr(blk, i):
                (k0, rows, m0, h, p) = UP_CHUNKS[i]
                t = xin.tile([128, CB, XW], BF16, name=f"xr{i}",
                             tag=f"xr{i}")
                nc.sync.dma_start(
                    out=t[:p * h], in_=xr_d[i][:, blk * CB:blk * CB + CB, :])
                xr_blk[blk][i] = t

            load_wu(2)
            # first-processed chunk's x in two channel-halves so the first
            # matmuls (channels 0-5) start ~1.7us earlier
            (k0_, rows_, m0_, h_, p_) = UP_CHUNKS[2]
            t0_ = xin.tile([128, CB, XW], BF16, name="xr2", tag="xr2")
            nc.sync.dma_start(out=t0_[:p_ * h_, 0:16],
                              in_=xr_d[2][:, 0:16, :])
            nc.sync.dma_start(out=t0_[:p_ * h_, 16:32],
                              in_=xr_d[2][:, 16:32, :])
            xr_blk[0][2] = t0_
            load_wu(0)
            load_xr(0, 0)
            load_wu(1)
            load_xr(0, 1)
            for i, (ci, K, p0, M, pk) in enumerate(DN_TILES):
                t = consts.tile([128, 2 * (6 // pk), 2, 128], FP8,
                                name=f"wd{i}", tag=f"wd{i}")
                nc.sync.dma_start(out=t[:K], in_=wd_d[i][:, :, :, :])
                wd_sb[i] = t
            for i in range(3):
                load_xr(1, i)

            pending_stores = []
            for blk in range(NBLK):
                ch0 = blk * CB
                # prefetch block+2 now: with bufs=3 its buffer-wait is
                # already satisfied, so it never parks the SP queue ahead
                # of this block's replica pieces
                if blk + 2 < NBLK:
                    for i in range(3):
                        load_xr(blk + 2, i)
                xr_sb = xr_blk[blk]

                # ---- up conv + P/N fp8 evacuation, per chunk ----
                # chunk 3 FIRST: its evacs finish early so the replica
                # pieces overlap the rest of the up phase instead of
                # stalling the down conv
                pn_sb = [None] * 3
                for i in (2, 0, 1):
                    (k0, rows, m0, h, pack) = UP_CHUNKS[i]
                    pn_t = pnp.tile([128, 2, 2, 133, CB], FP8,
                                    name=f"pn{i}", tag=f"pn{i}")
                    pn_sb[i] = pn_t
                    # junk col v=132 (Y cols 264/265): zero BEFORE the evacs
                    # so the chunk-3 replica pieces can copy it immediately
                    nc.gpsimd.memset(pn_t[:rows, :, :, 132:133, :], 0.0)
                    for b in (0, 1):
                        streams = _up_streams(b, pack)
                        for batch in _batches():
                            pu = pup.tile([128, 2, 512], F32, name="pu",
                                          tag="pu")
                            mrows = m_l[i]
                            for si, dv0 in enumerate(streams):
                                wsl = wu_sb[i][:pack * h, b, si, :mrows]
                                for gi, (c0, ncg) in enumerate(batch):
                                    nc.tensor.matmul(
                                        pu[:mrows, gi, 0:ncg * 132],
                                        wsl,
                                        xr_sb[i][:pack * h, c0:c0 + ncg,
                                                 5 + dv0:5 + dv0 + 132],
                                        start=(si == 0),
                                        stop=(si == len(streams) - 1))
                            # evacuate PSUM -> pn fp8 (P on ACT, N on DVE)
                            ngi = len(batch)
                            c0 = batch[0][0]
                            ncg = batch[0][1]
                            nct = sum(g[1] for g in batch)
                            pub = pu[:rows, 0, 0]
                            in_ap = _ap(pub, [list(pub.ap[0]),
                                              [1, 132], [512, ngi],
                                              [132, ncg]])
                            for slot in (0, 1):
                                ob = pn_t[0:rows, slot, b, 0, c0]
                                out_ap = _ap(ob, [list(ob.ap[0]),
                                                  [CB, 132], [ncg, ngi],
                                                  [1, ncg]])
                                if slot == 0:
                                    nc.scalar.activation(
                                        out=out_ap, in_=in_ap,
                                        func=mybir.ActivationFunctionType.Relu,
                                        scale=16.0)
                                elif ngi == 1:
                                    # small last batch: N on ACT too, to
                                    # debit the busier DVE
                                    nc.scalar.activation(
                                        out=out_ap, in_=in_ap,
                                        func=mybir.ActivationFunctionType.Relu,
                                        scale=-16.0)
                                else:
                                    nc.vector.tensor_scalar(
                                        out=out_ap, in0=in_ap,
                                        scalar1=-16.0, scalar2=0.0,
                                        op0=mybir.AluOpType.mult,
                                        op1=mybir.AluOpType.max)
                            if i == 2:
                                # v-shifted replica piece -> partitions
                                # [54,108): packs down col-tap pairs (dd,dd+3)
                                for slot in (0, 1):
                                    nc.sync.dma_start(
                                        out=pn_t[54:108, slot, b, 0:130,
                                                 c0:c0 + nct],
                                        in_=pn_t[0:54, slot, b, 3:133,
                                                 c0:c0 + nct])

                # ---- down conv (fp8 DoubleRow, slots = P/N) ----
                # All three row-tiles accumulate into ONE 128-partition PSUM
                # per q-chunk (stationary bands at column offset p0), so the
                # per-block store covers all 127 Z rows in one 128-partition
                # DMA (the v1 DMA cost is free-bytes-per-partition only).
                stg = stgp.tile([128, CB, 128], F32, name="st", tag="st")
                n_mm = sum(2 * (6 // pk) for (_, _, _, _, pk) in DN_TILES)
                for qc in range(8):
                    pd = pdn.tile([128, 512], F32, name="pd", tag="pd")
                    idx = 0
                    for ti, (ci, K, p0, M, pk) in enumerate(DN_TILES):
                        pn_t = pn_sb[ci]
                        nst = 6 // pk
                        for bp in (0, 1):
                            for dd in range(nst):
                                wsl = wd_sb[ti][:K, bp * nst + dd, :, :]
                                mb_ = pn_t[0:K, 0, bp, 16 * qc + dd, 0]
                                rhs = _ap(mb_, [list(mb_.ap[0]),
                                                [2 * 133 * CB, 2],
                                                [1, 16 * CB]])
                                nc.tensor.matmul(
                                    pd[:128, :], wsl, rhs,
                                    start=(idx == 0), stop=(idx == n_mm - 1),
                                    perf_mode=mybir.MatmulPerfMode.DoubleRow)
                                idx += 1
                    # permute (q,c)->(c,q), scale 2^-14, PSUM -> SBUF.
                    # ACT (idle during the down phase): GPSIMD cannot read
                    # PSUM on hardware.
                    pb = pd[:QO, 0]
                    in_ap = _ap(pb, [list(pb.ap[0]), [1, CB], [CB, 16]])
                    nc.scalar.activation(
                        out=stg[:QO, :, 16 * qc:16 * qc + 16],
                        in_=in_ap, func=mybir.ActivationFunctionType.Copy,
                        scale=OUT_SCALE)
                # defer this block's stores until after the NEXT block's
                # up phase has emitted its replica pieces: the stores' stg
                # waits then never park the SP queue ahead of the pieces.
                # q-split so earlier stores overlap the rest of the down conv.
                def emit_stores(ch0=ch0, stg=stg):
                    for (qa, qb) in ((0, 64), (64, 96), (96, 112),
                                     (112, QO)):
                        nc.sync.dma_start(
                            out=out_d[ch0:ch0 + CB, :, qa:qb]
                            .rearrange("c p q -> p c q"),
                            in_=stg[:QO, :, qa:qb])
                pending_stores.append(emit_stores)
                if blk == NBLK - 1:
                    for es in pending_stores:
                        es()
                    pending_stores = []
    nc.compile()
    return nc


_CACHE = {}


def _host_build(input, bias, up_filter, down_filter):
    input = np.asarray(input, dtype=np.float32)
    bias = np.asarray(bias, dtype=np.float32)
    fu = np.asarray(up_filter, np.float32)
    fd = np.asarray(down_filter, np.float32)
    wu = _build_up_weights(fu)
    wd = _build_dn_weights(fd)
    in_maps = []
    for i in range(N_CORES):
        xbi = input[i] + bias[:, None, None]
        xrs = _build_x_replicas(xbi)
        m = {f"xr{j}": xrs[j] for j in range(3)}
        m.update({f"wu{j}": wu[j] for j in range(3)})
        m.update({f"wd{j}": wd[j] for j in range(3)})
        in_maps.append(m)
    return in_maps


def kernel(input, bias, up_filter, down_filter):
    in_maps = _host_build(input, bias, up_filter, down_filter)
    if "nc" not in _CACHE:
        _CACHE["nc"] = _build_program()
    nc = _CACHE["nc"]
    res = run_bass_kernel_spmd(nc, in_maps, core_ids=list(range(N_CORES)))
    globals()["_LAST_RESULT"] = res
    return np.stack([r["out"] for r in res.results], axis=0)


if __name__ == "__main__":
    rng = np.random.default_rng(0)
    out = kernel(rng.standard_normal((8, C, H, W), dtype=np.float32),
                 rng.standard_normal((C,), dtype=np.float32),
                 rng.random((12, 12), dtype=np.float32),
                 rng.random((12, 12), dtype=np.float32))
    print(out.shape, out.dtype)


# revision 48
# speedup vs baseline: 1.1118x; 1.1118x over previous
"""Alias-free activation (StyleGAN3-style) Trainium2 Bass kernel, v2.

Pipeline per image: bias-add -> 2x zero-insert upsample + 12x12 FIR (pad 10,
gain 4) -> leaky-relu(0.2)*sqrt(2) [clamp +-256 provably inactive: |Y|<=5.1]
-> 12x12 FIR + 2x downsample.

Strategy (pure data parallel, 8 images -> 8 cores):
- UP conv on TensorE as banded-Toeplitz matmuls over x rows, bf16, with TWO
  (tail: THREE) column taps packed per stream via host-prebuilt col-shifted
  replica tensors stacked along the contraction partitions. Bias and the
  4*sqrt(2)*up^2 gain are folded host-side (into x replicas / up weights).
- Leaky-relu is algebraically split: P = Relu(16*yhat) on ScalarE and
  N = Relu(-16*yhat) ((x*-16) max 0) on VectorE, both cast to fp8e4m3 and
  written interleaved as the two DoubleRow slots of the down conv.
- DOWN conv on TensorE in fp8 DoubleRow (0.5 cyc/row): slot0 multiplies P by
  1024*fd, slot1 multiplies N by -0.2*1024*fd; all three row-tiles place
  their bands at stationary column offset p0 and accumulate into ONE
  128-partition PSUM per 16-col q-chunk, so each block needs a single
  128-partition store (q-split for overlap). The down tail packs col-tap
  pairs (dd, dd+3) via a v-shifted SBUF replica of the chunk-3 P/N tile.
  ACT evacuates PSUM->SBUF with a 2^-14 output scale (GPSIMD cannot touch
  PSUM on hardware). Per block, chunk 3 is computed first so its replica
  pieces overlap the rest of the up phase. Measured end-to-end rel err
  ~9.5e-3 on hardware (gate 2e-2).
"""
import numpy as np
import ml_dtypes

import concourse.bass as bass
import concourse.bacc as bacc
import concourse.tile as tile
from concourse import mybir
from concourse.bass_utils import run_bass_kernel_spmd

N_CORES = 8
C, H, W = 128, 128, 128
QO = 127            # output spatial size
CB = 32             # channels per block
NBLK = C // CB
CG = 3              # channels per up matmul group (3*132 = 396 <= 512 psum)
A_GAIN = 4.0 * float(np.sqrt(2.0))   # up^2 * leaky sqrt(2) gain
SW = 1024.0          # down fp8 weight scale (power of 2)
OUT_SCALE = 1.0 / (16.0 * SW)
XW = 138            # padded x width (5 each side)

# (k0, rows, m0, h, pack): up chunk of interleaved Y rows [k0, k0+rows),
# contracting x rows [m0, m0+h), `pack` col-taps stacked on partitions.
UP_CHUNKS = [
    (0, 116, 0, 59, 2),
    (106, 116, 48, 64, 2),
    (210, 54, 100, 28, 3),
]
# (ci, K, p0, M, pk): down tile reading pn chunk ci rows [0, K), writing Z
# rows [p0, p0+M).  t0 = UP_CHUNKS[ci].k0.  pk=2: col-taps (dd, dd+3) are
# partition-packed via the v-shifted replica in partitions [K/2, K).
DN_TILES = [
    (0, 116, 0, 53, 1),
    (1, 114, 53, 52, 1),
    (2, 108, 105, 22, 2),
]
F32 = mybir.dt.float32
F32R = mybir.dt.float32r
BF16 = mybir.dt.bfloat16
FP8 = mybir.dt.float8e4
NP_BF16 = ml_dtypes.bfloat16
NP_E4 = ml_dtypes.float8_e4m3


def _up_streams(b, pack):
    base = -5 if b == 0 else -4
    return list(range(base, base + 6, pack))


def _batches():
    """Channel-group batches within a block: [[(c0, ncg), (c0, ncg)], ...]."""
    groups = []
    c0 = 0
    while c0 < CB:
        groups.append((c0, min(CG, CB - c0)))
        c0 += CG
    return [groups[i:i + 2] for i in range(0, len(groups), 2)]


def _build_up_weights(fu):
    """Per-chunk stationary banks [pack*h, 2, ns, rows] (bf16, gain folded)."""
    fua = fu.astype(np.float64) * A_GAIN
    out = []
    for (k0, rows, m0, h, pack) in UP_CHUNKS:
        ns = len(_up_streams(0, pack))
        w = np.zeros((pack * h, 2, ns, rows), np.float64)
        kl = np.arange(rows)[None, :]
        rr = np.arange(h)[:, None]
        frow = 2 * (m0 + rr) - (k0 + kl) + 10      # [h, rows]
        fvalid = (frow >= 0) & (frow <= 11)
        frc = np.clip(frow, 0, 11)
        for b in (0, 1):
            for si, dv0 in enumerate(_up_streams(b, pack)):
                for cp in range(pack):
                    fcol = 2 * (dv0 + cp) - b + 10
                    assert 0 <= fcol <= 11
                    w[cp * h:(cp + 1) * h, b, si, :] = np.where(
                        fvalid, fua[frc, fcol], 0.0)
        out.append(w.astype(NP_BF16))
    return out


def _build_dn_weights(fd):
    """Per-tile DoubleRow banks [K, 12, 2, 128] (fp8): slot0 = SW*fd,
    slot1 = -0.2*SW*fd for the N = relu(-yhat) branch.  The band sits at
    stationary columns [p0, p0+M) so all three tiles accumulate into one
    128-partition PSUM (Z row p = partition p)."""
    fdq = fd.astype(np.float64)
    out = []
    for (ci, K, p0, M, pk) in DN_TILES:
        t0 = UP_CHUNKS[ci][0]
        kh = 54 if pk == 2 else K
        cp_base = [0, 54]
        nst = 6 // pk
        w = np.zeros((K, 2 * nst, 2, 128), np.float64)
        r = np.arange(kh)[:, None]
        pg = np.arange(p0, p0 + M)[None, :]
        dr = (t0 + r) - 2 * pg                     # [kh, M]
        valid = (dr >= 0) & (dr <= 11)
        drc = np.clip(dr, 0, 11)
        for bp in (0, 1):
            for dd in range(nst):
                idx = bp * nst + dd
                for cp in range(pk):
                    b0_ = cp_base[cp] if pk == 2 else 0
                    band = np.where(valid, fdq[drc, 2 * (dd + 3 * cp) + bp],
                                    0.0)
                    w[b0_:b0_ + kh, idx, 0, p0:p0 + M] = SW * band
                    w[b0_:b0_ + kh, idx, 1, p0:p0 + M] = -0.2 * SW * band
        out.append(w.astype(NP_E4))
    return out


def _build_x_replicas(xb):
    """xb: [C, H, W] fp32 bias-folded image -> per-chunk replica tensors
    [pack*h, C, XW] bf16; copy cp holds x rows shifted by cp columns."""
    out = []
    for (k0, rows, m0, h, pack) in UP_CHUNKS:
        xr = np.zeros((pack * h, C, XW), np.float32)
        src = np.transpose(xb[:, m0:m0 + h, :], (1, 0, 2))  # [h, C, W]
        for cp in range(pack):
            xr[cp * h:(cp + 1) * h, :, 5 - cp:133 - cp] = src
        out.append(xr.astype(NP_BF16))
    return out


def _ap(base, dims):
    """Manual AP with base's tensor/offset and explicit [stride, count] dims."""
    return bass.AP(tensor=base.tensor, offset=base.offset, ap=dims)


def _build_program() -> bacc.Bacc:
    nc = bacc.Bacc("TRN2", target_bir_lowering=False, debug=False,
                   num_devices=N_CORES)
    xr_d = [nc.dram_tensor(f"xr{i}", [p * h, C, XW], BF16,
                           kind="ExternalInput")
            for i, (k0, rows, m0, h, p) in enumerate(UP_CHUNKS)]
    ns_l = [len(_up_streams(0, p)) for (_, _, _, _, p) in UP_CHUNKS]
    m_l = [rows for (k0, rows, m0, h, p) in UP_CHUNKS]
    wu_d = [nc.dram_tensor(f"wu{i}", [p * h, 2, ns_l[i], m_l[i]], BF16,
                           kind="ExternalInput")
            for i, (k0, rows, m0, h, p) in enumerate(UP_CHUNKS)]
    wd_d = [nc.dram_tensor(f"wd{i}", [K, 2 * (6 // pk), 2, 128], FP8,
                           kind="ExternalInput")
            for i, (ci, K, p0, M, pk) in enumerate(DN_TILES)]
    out_d = nc.dram_tensor("out", [C, QO, QO], F32, kind="ExternalOutput")

    with tile.TileContext(nc) as tc:
        with (
            tc.tile_pool(name="consts", bufs=1) as consts,
            tc.tile_pool(name="xin", bufs=2) as xin,
            tc.tile_pool(name="pn", bufs=2) as pnp,
            tc.tile_pool(name="stg", bufs=1) as stgp,
            tc.tile_pool(name="pup", bufs=3, space="PSUM") as pup,
            tc.tile_pool(name="pdn", bufs=2, space="PSUM") as pdn,
        ):
            # first chunk's input + weights first so PE starts ASAP; then the
            # rest. All loads upfront: the SP queue then only ever waits on
            # xin buffer recycling (prefetch), never behind the out-store
            # DMAs' stg waits.
            wu_sb = [None] * 3
            wd_sb = [None] * 3
            xr_blk = [[None] * 3 for _ in range(NBLK)]

            def load_wu(i):
                (k0, rows, m0, h, p) = UP_CHUNKS[i]
                t = consts.tile([128, 2, ns_l[i], m_l[i]], BF16,
                                name=f"wu{i}", tag=f"wu{i}")
                nc.sync.dma_start(out=t[:p * h], in_=wu_d[i][:, :, :, :])
                wu_sb[i] = t

            def load_xr(blk, i):
                (k0, rows, m0, h, p) = UP_CHUNKS[i]
                t = xin.tile([128, CB, XW], BF16, name=f"xr{i}",
                             tag=f"xr{i}")
                nc.sync.dma_start(
                    out=t[:p * h], in_=xr_d[i][:, blk * CB:blk * CB + CB, :])
                xr_blk[blk][i] = t

            load_wu(2)
            # first-processed chunk's x in two channel-halves so the first
            # matmuls (channels 0-5) start ~1.7us earlier
            (k0_, rows_, m0_, h_, p_) = UP_CHUNKS[2]
            t0_ = xin.tile([128, CB, XW], BF16, name="xr2", tag="xr2")
            nc.sync.dma_start(out=t0_[:p_ * h_, 0:16],
                              in_=xr_d[2][:, 0:16, :])
            nc.sync.dma_start(out=t0_[:p_ * h_, 16:32],
                              in_=xr_d[2][:, 16:32, :])
            xr_blk[0][2] = t0_
            load_wu(0)
            load_xr(0, 0)
            load_wu(1)
            load_xr(0, 1)
            for i, (ci, K, p0, M, pk) in enumerate(DN_TILES):
                t = consts.tile([128, 2 * (6 // pk), 2, 128], FP8,
                                name=f"wd{i}", tag=f"wd{i}")
                nc.sync.dma_start(out=t[:K], in_=wd_d[i][:, :, :, :])
                wd_sb[i] = t
            for i in range(3):
                load_xr(1, i)

            blocks_pn = {}
            SENT = object()

            def gen_up(blk):
                ch0 = blk * CB
                if blk + 1 < NBLK and xr_blk[blk + 1][0] is None:
                    for i in range(3):
                        load_xr(blk + 1, i)
                xr_sb = xr_blk[blk]
                # chunk 3 FIRST: its evacs finish early so the replica
                # pieces are ready well before down(blk) needs them
                pn_sb = [None] * 3
                for i in (2, 0, 1):
                    (k0, rows, m0, h, pack) = UP_CHUNKS[i]
                    pn_t = pnp.tile([128, 2, 2, 133, CB], FP8,
                                    name=f"pn{i}", tag=f"pn{i}")
                    pn_sb[i] = pn_t
                    nc.gpsimd.memset(pn_t[:rows, :, :, 132:133, :], 0.0)
                blocks_pn[blk] = pn_sb
                for i in (2, 0, 1):
                    (k0, rows, m0, h, pack) = UP_CHUNKS[i]
                    pn_t = pn_sb[i]
                    for b in (0, 1):
                        streams = _up_streams(b, pack)
                        for batch in _batches():
                            pu = pup.tile([128, 2, 512], F32, name="pu",
                                          tag="pu")
                            mrows = m_l[i]
                            for si, dv0 in enumerate(streams):
                                wsl = wu_sb[i][:pack * h, b, si, :mrows]
                                for gi, (c0, ncg) in enumerate(batch):
                                    nc.tensor.matmul(
                                        pu[:mrows, gi, 0:ncg * 132],
                                        wsl,
                                        xr_sb[i][:pack * h, c0:c0 + ncg,
                                                 5 + dv0:5 + dv0 + 132],
                                        start=(si == 0),
                                        stop=(si == len(streams) - 1))
                            # evacuate PSUM -> pn fp8 (P on ACT, N on DVE)
                            ngi = len(batch)
                            c0 = batch[0][0]
                            ncg = batch[0][1]
                            nct = sum(g[1] for g in batch)
                            pub = pu[:rows, 0, 0]
                            in_ap = _ap(pub, [list(pub.ap[0]),
                                              [1, 132], [512, ngi],
                                              [132, ncg]])
                            for slot in (0, 1):
                                ob = pn_t[0:rows, slot, b, 0, c0]
                                out_ap = _ap(ob, [list(ob.ap[0]),
                                                  [CB, 132], [ncg, ngi],
                                                  [1, ncg]])
                                if slot == 0:
                                    nc.scalar.activation(
                                        out=out_ap, in_=in_ap,
                                        func=mybir.ActivationFunctionType.Relu,
                                        scale=16.0)
                                elif ngi == 1:
                                    # small last batch: N on ACT too, to
                                    # debit the busier DVE
                                    nc.scalar.activation(
                                        out=out_ap, in_=in_ap,
                                        func=mybir.ActivationFunctionType.Relu,
                                        scale=-16.0)
                                else:
                                    nc.vector.tensor_scalar(
                                        out=out_ap, in0=in_ap,
                                        scalar1=-16.0, scalar2=0.0,
                                        op0=mybir.AluOpType.mult,
                                        op1=mybir.AluOpType.max)
                            if i == 2:
                                # v-shifted replica piece -> partitions
                                # [54,108): packs down col-tap pairs (dd,dd+3)
                                for slot in (0, 1):
                                    nc.sync.dma_start(
                                        out=pn_t[54:108, slot, b, 0:130,
                                                 c0:c0 + nct],
                                        in_=pn_t[0:54, slot, b, 3:133,
                                                 c0:c0 + nct])
                        yield

            def gen_down(blk):
                # All three row-tiles accumulate into ONE 128-partition PSUM
                # per q-chunk (stationary bands at column offset p0), so the
                # per-block store covers all 127 Z rows in one 128-partition
                # DMA (the v1 DMA cost is free-bytes-per-partition only).
                ch0 = blk * CB
                pn_sb = blocks_pn[blk]
                stg = stgp.tile([128, CB, 128], F32, name="st", tag="st")
                n_mm = sum(2 * (6 // pk) for (_, _, _, _, pk) in DN_TILES)
                for qc in range(8):
                    pd = pdn.tile([128, 512], F32, name="pd", tag="pd")
                    idx = 0
                    for ti, (ci, K, p0, M, pk) in enumerate(DN_TILES):
                        pn_t = pn_sb[ci]
                        nst = 6 // pk
                        for bp in (0, 1):
                            for dd in range(nst):
                                wsl = wd_sb[ti][:K, bp * nst + dd, :, :]
                                mb_ = pn_t[0:K, 0, bp, 16 * qc + dd, 0]
                                rhs = _ap(mb_, [list(mb_.ap[0]),
                                                [2 * 133 * CB, 2],
                                                [1, 16 * CB]])
                                nc.tensor.matmul(
                                    pd[:128, :], wsl, rhs,
                                    start=(idx == 0), stop=(idx == n_mm - 1),
                                    perf_mode=mybir.MatmulPerfMode.DoubleRow)
                                idx += 1
                    # permute (q,c)->(c,q), scale 2^-14, PSUM -> SBUF.
                    # ACT: GPSIMD cannot read PSUM on hardware.
                    pb = pd[:QO, 0]
                    in_ap = _ap(pb, [list(pb.ap[0]), [1, CB], [CB, 16]])
                    nc.scalar.activation(
                        out=stg[:QO, :, 16 * qc:16 * qc + 16],
                        in_=in_ap, func=mybir.ActivationFunctionType.Copy,
                        scale=OUT_SCALE)
                    yield
                # q-split so earlier stores overlap the remaining work
                for (qa, qb) in ((0, 64), (64, 96), (96, 112),
                                 (112, QO)):
                    nc.sync.dma_start(
                        out=out_d[ch0:ch0 + CB, :, qa:qb]
                        .rearrange("c p q -> p c q"),
                        in_=stg[:QO, :, qa:qb])
                yield

            # Interleave up(b) with down(b-1) on the PE queue: the down
            # matmul groups fill the PSUM-recycle gaps of the evac-paced
            # up phase, and the evac engines get the whole period to drain.
            for _ in gen_up(0):
                pass
            for blk in range(1, NBLK):
                gu, gd = gen_up(blk), gen_down(blk - 1)
                u_done = d_done = False
                while not (u_done and d_done):
                    if not u_done:
                        u_done = next(gu, SENT) is SENT
                    if not d_done:
                        d_done = next(gd, SENT) is SENT
            for _ in gen_down(NBLK - 1):
                pass
    nc.compile()
    return nc


_CACHE = {}


def _host_build(input, bias, up_filter, down_filter):
    input = np.asarray(input, dtype=np.float32)
    bias = np.asarray(bias, dtype=np.float32)
    fu = np.asarray(up_filter, np.float32)
    fd = np.asarray(down_filter, np.float32)
    wu = _build_up_weights(fu)
    wd = _build_dn_weights(fd)
    in_maps = []
    for i in range(N_CORES):
        xbi = input[i] + bias[:, None, None]
        xrs = _build_x_replicas(xbi)
        m = {f"xr{j}": xrs[j] for j in range(3)}
        m.update({f"wu{j}": wu[j] for j in range(3)})
        m.update({f"wd{j}": wd[j] for j in range(3)})
        in_maps.append(m)
    return in_maps


def kernel(input, bias, up_filter, down_filter):
    in_maps = _host_build(input, bias, up_filter, down_filter)
    if "nc" not in _CACHE:
        _CACHE["nc"] = _build_program()
    nc = _CACHE["nc"]
    res = run_bass_kernel_spmd(nc, in_maps, core_ids=list(range(N_CORES)))
    globals()["_LAST_RESULT"] = res
    return np.stack([r["out"] for r in res.results], axis=0)


if __name__ == "__main__":
    rng = np.random.default_rng(0)
    out = kernel(rng.standard_normal((8, C, H, W), dtype=np.float32),
                 rng.standard_normal((C,), dtype=np.float32),
                 rng.random((12, 12), dtype=np.float32),
                 rng.random((12, 12), dtype=np.float32))
    print(out.shape, out.dtype)
